# revision 1
# baseline (speedup 1.0000x reference)
"""AC-loss (argmax-coords + l2) kernel for 16x64x256x256 inputs on 8 TRN2
NeuronCores, data-parallel over the batch.

Per core (2 samples = 128 (b,c) rows x 65536 pixels, for predict and gt):
  - per-row argmax via 512-wide window maxes (one windowed DVE tensor_reduce
    per streamed chunk) + one tiny indirect-DMA refetch of each row's
    winning 512-element window,
  - per-row sum((p-g)^2) via GpSimd subtract + ScalarE Square-accumulate.

Streaming uses uniform 2048-element chunks (no separate ramp phase) with
all stream DMAs issued from the SP HWDGE queue; compute-engine queues
stay free of DMA issues so buffer-wait stalls never block compute.

Host combines: coords -> pairwise-distance/angle MSEs -> w_ac, l2 -> loss.
"""
from contextlib import ExitStack

import numpy as np

import concourse.bass as bass
import concourse.tile as tile
from concourse import bacc, mybir
from concourse.bass_utils import run_bass_kernel_spmd

F32 = mybir.dt.float32
I32 = mybir.dt.int32
I16 = mybir.dt.int16
BF16 = mybir.dt.bfloat16
P = 128

# problem shape (hardcoded per spec)
B, C, H, W = 16, 64, 256, 256
HW = H * W
N_CORES = 8
BPC = B // N_CORES          # samples per core
K = 2048                    # streaming chunk width
NCH = HW // K               # 32 chunks
WIN = 512                   # argmax window width
NW = HW // WIN              # 128 windows per row
WPC = K // WIN              # 4 windows per chunk
IDX_OFFSET = (NW + 1) * WIN  # device indices are shifted by -(NW+1)*WIN

EPS_ACOS = 1e-7
EPS_COS = 1e-8


SUB = 2      # chunk 0 split into SUB pieces to shorten the ramp
PIECE = K // SUB  # ramp piece width (1024 = 2 windows)


def _build_nc(io_bufs=8):
    nc = bacc.Bacc("TRN2", target_bir_lowering=False, debug=False,
                   num_devices=N_CORES)
    p_dram = nc.declare_dram_parameter("p", [P, HW], F32, isOutput=False)
    g_dram = nc.declare_dram_parameter("g", [P, HW], F32, isOutput=False)
    out_dram = nc.declare_dram_parameter("out3", [P, 3], F32, isOutput=True)

    with tile.TileContext(nc) as tc, ExitStack() as ctx:
        io = ctx.enter_context(tc.tile_pool(name="io", bufs=io_bufs))
        ramp = ctx.enter_context(tc.tile_pool(name="ramp", bufs=SUB))
        pg_w = ctx.enter_context(tc.tile_pool(name="pg_w", bufs=2))
        act_w = ctx.enter_context(tc.tile_pool(name="act_w", bufs=2))
        singles = ctx.enter_context(tc.tile_pool(name="singles", bufs=1))

        maxp = singles.tile([P, NW], F32)
        maxg = singles.tile([P, NW], F32)
        l2c = singles.tile([P, NCH + SUB - 1], F32)

        # tail constants (all tiny): within-window iota j-WIN, window iota
        # w-NW, per-row base row*NW + NW
        iota_w = singles.tile([P, WIN], I16)
        nc.gpsimd.iota(iota_w[:], pattern=[[1, WIN]], base=-WIN,
                       channel_multiplier=0)
        iota_nw = singles.tile([P, NW], F32)
        nc.gpsimd.iota(iota_nw[:], pattern=[[1, NW]], base=-NW,
                       channel_multiplier=0,
                       allow_small_or_imprecise_dtypes=True)
        prowB = singles.tile([P, 1], F32)
        nc.gpsimd.iota(prowB[:], pattern=[[0, 1]], base=NW,
                       channel_multiplier=NW,
                       allow_small_or_imprecise_dtypes=True)

        # Issue chunk 1's big DMAs FIRST: the stream is DMA-bound, so DMA
        # bandwidth must saturate immediately; the small ramp pieces below
        # only exist to start engine work early and their serialized issue
        # must not delay the bulk stream.
        pt1 = io.tile([P, K], F32, tag="p")
        nc.sync.dma_start(out=pt1[:], in_=p_dram[:, K:2 * K])
        gt1 = io.tile([P, K], F32, tag="g")
        nc.sync.dma_start(out=gt1[:], in_=g_dram[:, K:2 * K])

        # ramp: chunk 0 as SUB pieces so the first compute starts as soon
        # as ~1MB (not 2MB) has landed
        WPP = PIECE // WIN
        for s in range(SUB):
            pt = ramp.tile([P, PIECE], F32, tag="rp")
            nc.sync.dma_start(out=pt[:],
                              in_=p_dram[:, s * PIECE:(s + 1) * PIECE])
            gt_ = ramp.tile([P, PIECE], F32, tag="rg")
            nc.sync.dma_start(out=gt_[:],
                              in_=g_dram[:, s * PIECE:(s + 1) * PIECE])
            nc.vector.tensor_reduce(
                out=maxp[:, s * WPP:(s + 1) * WPP],
                in_=pt[:].rearrange("p (w k) -> p w k", k=WIN),
                axis=mybir.AxisListType.X, op=mybir.AluOpType.max)
            nc.vector.tensor_reduce(
                out=maxg[:, s * WPP:(s + 1) * WPP],
                in_=gt_[:].rearrange("p (w k) -> p w k", k=WIN),
                axis=mybir.AxisListType.X, op=mybir.AluOpType.max)
            wd = pg_w.tile([P, K], BF16, tag="wd")
            nc.gpsimd.tensor_tensor(
                out=wd[:, :PIECE], in0=pt[:], in1=gt_[:],
                op=mybir.AluOpType.subtract)
            a0 = act_w.tile([P, K], BF16, tag="actw")
            nc.scalar.activation(
                out=a0[:, :PIECE], in_=wd[:, :PIECE],
                func=mybir.ActivationFunctionType.Square,
                accum_out=l2c[:, s:s + 1])

        for c in range(1, NCH):
            if c == 1:
                pt, gt_ = pt1, gt1
            else:
                pt = io.tile([P, K], F32, tag="p")
                nc.sync.dma_start(out=pt[:],
                                  in_=p_dram[:, c * K:(c + 1) * K])
                gt_ = io.tile([P, K], F32, tag="g")
                nc.sync.dma_start(out=gt_[:],
                                  in_=g_dram[:, c * K:(c + 1) * K])
            nc.vector.tensor_reduce(
                out=maxp[:, c * WPC:(c + 1) * WPC],
                in_=pt[:].rearrange("p (w k) -> p w k", k=WIN),
                axis=mybir.AxisListType.X, op=mybir.AluOpType.max)
            nc.vector.tensor_reduce(
                out=maxg[:, c * WPC:(c + 1) * WPC],
                in_=gt_[:].rearrange("p (w k) -> p w k", k=WIN),
                axis=mybir.AxisListType.X, op=mybir.AluOpType.max)
            wd = pg_w.tile([P, K], BF16, tag="wd")
            nc.gpsimd.tensor_tensor(
                out=wd[:], in0=pt[:], in1=gt_[:],
                op=mybir.AluOpType.subtract)
            a0 = act_w.tile([P, K], BF16, tag="actw")
            nc.scalar.activation(
                out=a0[:], in_=wd[:],
                func=mybir.ActivationFunctionType.Square,
                accum_out=l2c[:, SUB - 1 + c:SUB + c])

        out3 = singles.tile([P, 3], F32)
        nc.vector.tensor_reduce(out=out3[:, 2:3], in_=l2c[:],
                                axis=mybir.AxisListType.X,
                                op=mybir.AluOpType.add)

        # Tails are split into pre (window-find + refetch issue) and post
        # (scan) phases, emitted p-pre, g-pre, p-post, g-post: the in-order
        # DVE queue would otherwise park g's ready pre-work behind p's
        # refetch wait, leaving DVE idle for the full refetch latency.
        def tail_pre(dram, cols, tag):
            # device idx' = (fw-NW)*WIN + (li-WIN); host adds (NW+1)*WIN back
            gmax = singles.tile([P, 1], F32, tag=f"gmax_{tag}")
            nc.vector.tensor_reduce(out=gmax[:], in_=cols[:],
                                    axis=mybir.AxisListType.X,
                                    op=mybir.AluOpType.max)
            valc = singles.tile([P, NW], F32, tag=f"valc_{tag}")
            nc.vector.scalar_tensor_tensor(
                out=valc[:], in0=cols[:], scalar=gmax[:, 0:1], in1=iota_nw[:],
                op0=mybir.AluOpType.is_equal, op1=mybir.AluOpType.mult)
            fw = singles.tile([P, 1], F32, tag=f"fw_{tag}")
            nc.vector.tensor_reduce(out=fw[:], in_=valc[:],
                                    axis=mybir.AxisListType.X,
                                    op=mybir.AluOpType.min)
            rowi = singles.tile([P, 1], I32, tag=f"rowi_{tag}")
            nc.vector.tensor_scalar(
                out=rowi[:], in0=prowB[:], scalar1=fw[:], scalar2=None,
                op0=mybir.AluOpType.add)
            win = singles.tile([P, WIN], F32, tag=f"win_{tag}")
            nc.gpsimd.indirect_dma_start(
                out=win[:], out_offset=None,
                in_=dram[:].rearrange("a (b k) -> (a b) k", k=WIN),
                in_offset=bass.IndirectOffsetOnAxis(ap=rowi[:, :1], axis=0))
            return gmax, fw, win

        def tail_post(gmax, fw, win, out_col, tag):
            valw = singles.tile([P, WIN], I16, tag=f"valw_{tag}")
            nc.vector.scalar_tensor_tensor(
                out=valw[:], in0=win[:], scalar=gmax[:, 0:1], in1=iota_w[:],
                op0=mybir.AluOpType.is_equal, op1=mybir.AluOpType.mult)
            li = singles.tile([P, 1], F32, tag=f"li_{tag}")
            wmin = singles.tile([P, WIN], BF16, tag=f"wmin_{tag}")
            nc.vector.tensor_scalar(
                out=wmin[:], in0=valw[:], scalar1=0.0, scalar2=None,
                op0=mybir.AluOpType.bypass, op1=mybir.AluOpType.min,
                accum_out=li[:])
            nc.vector.scalar_tensor_tensor(
                out=out3[:, out_col:out_col + 1], in0=fw[:], scalar=float(WIN),
                in1=li[:],
                op0=mybir.AluOpType.mult, op1=mybir.AluOpType.add)

        hp = tail_pre(p_dram, maxp, "p")
        hg = tail_pre(g_dram, maxg, "g")
        tail_post(*hp, 0, "p")
        tail_post(*hg, 1, "g")
        nc.sync.dma_start(out=out_dram[:], in_=out3[:])

    nc.compile()
    return nc


_NC_CACHE = None


def _get_nc():
    global _NC_CACHE
    if _NC_CACHE is None:
        _NC_CACHE = _build_nc()
    return _NC_CACHE


# ---------------- host-side loss combination (mirrors the reference) -------

def _coords(idx):
    r = (idx // W).astype(np.float32)
    c = (idx % W).astype(np.float32)
    return np.stack([r, c], axis=-1)


def _pairwise_dist(xy):
    diff = xy[:, :, None, :] - xy[:, None, :, :]
    return np.sqrt((diff * diff).sum(axis=-1))


def _angle_matrix(xy):
    dots = np.einsum('bic,bjc->bij', xy, xy)
    norms = np.sqrt((xy * xy).sum(axis=-1))
    denom = np.maximum(norms[:, :, None] * norms[:, None, :],
                       np.float32(EPS_COS))
    cos = np.clip(dots / denom, np.float32(-1.0 + EPS_ACOS),
                  np.float32(1.0 - EPS_ACOS))
    return np.arccos(cos)


def _combine(pidx, gidx, l2row):
    p_xy = _coords(pidx)
    g_xy = _coords(gidx)
    dD = _pairwise_dist(p_xy) - _pairwise_dist(g_xy)
    mse_D = (dD * dD).mean(axis=(1, 2), dtype=np.float32)
    dA = _angle_matrix(p_xy) - _angle_matrix(g_xy)
    mse_A = (dA * dA).mean(axis=(1, 2), dtype=np.float32)
    w_ac = np.log2(mse_D) + np.log2(mse_A)
    l2 = l2row.sum(axis=1, dtype=np.float32) / np.float32(C * H * W)
    return np.float32((w_ac * l2).sum(dtype=np.float32) / np.float32(B))


def kernel(predict, gt):
    predict = np.ascontiguousarray(np.asarray(predict, dtype=np.float32))
    gt = np.ascontiguousarray(np.asarray(gt, dtype=np.float32))
    assert predict.shape == (B, C, H, W) and gt.shape == (B, C, H, W)

    in_maps = []
    for i in range(N_CORES):
        in_maps.append({
            "p": predict[i * BPC:(i + 1) * BPC].reshape(BPC * C, HW),
            "g": gt[i * BPC:(i + 1) * BPC].reshape(BPC * C, HW),
        })

    nc = _get_nc()
    res = run_bass_kernel_spmd(nc, in_maps, core_ids=list(range(N_CORES)))

    pidx = np.zeros((B, C), dtype=np.int64)
    gidx = np.zeros((B, C), dtype=np.int64)
    l2row = np.zeros((B, C), dtype=np.float32)
    for i in range(N_CORES):
        o = res.results[i]["out3"]
        pidx[i * BPC:(i + 1) * BPC] = (
            np.rint(o[:, 0].reshape(BPC, C)).astype(np.int64) + IDX_OFFSET)
        gidx[i * BPC:(i + 1) * BPC] = (
            np.rint(o[:, 1].reshape(BPC, C)).astype(np.int64) + IDX_OFFSET)
        l2row[i * BPC:(i + 1) * BPC] = o[:, 2].reshape(BPC, C)

    return np.asarray(_combine(pidx, gidx, l2row), dtype=np.float32)

